# revision 50
# baseline (speedup 1.0000x reference)
"""Trainium2 Bass kernel for nn_BondMatrixMessage (GNN bond-matrix message passing).

Per batch b (one NeuronCore each, B=8 => 8 cores):
    bw[e,(i,j)] = sum_k bond[e,k] * W[k,(i,j)]          (PE, bf16)
    m[e,i]      = sum_j bw[e,(i,j)] * atom[src[e],j]    (evac/mult + PE j-reduce)
    out[t,:]    = sum_{e: tgt[e]=t} m[e,:]              (sorted-edge scatter-add)

Feature-major chunked layout: per 512-edge tile, 8 chunks of 128 partitions,
chunk c partition p <-> (i = 4c + p%4, j = p//4).
  - srcg[p, e] = atom[src[e], p//4] is host-prepared (pure index/layout prep,
    same class as the host edge sort) and DMA-streamed per tile.
  - bw_c = W2_c^T @ bondT_tile (PSUM fp32, one 512-col matmul per chunk).
  - multiply route per chunk, alternating per tile parity (route/route2) to
    balance engines: 'A' = ACT-evac to SBUF bf16 then DVE tensor_tensor (2x
    mode); 'D' = DVE tensor_tensor direct from PSUM (1x); 'G' = ACT-evac +
    Pool tensor_tensor (GPSIMD cannot access PSUM).
  - j-reduction as weight-stationary matmuls: for each 128-edge block b,
    lhsT = pt_c[:, b*128:(b+1)*128] (the multiplied products as PE weights),
    rhs = sel_c (128, 32) with sel_c[p, m] = [4c + p%4 == m]:
    mt[e_local, i] += sum_p pt[p, e]*sel[p, i].  The 8 chunk matmuls of a
    block accumulate into PSUM CONSECUTIVELY (interleaved accumulation groups
    are miscompiled).  Output partition = token index mod 128 == the
    scatter's token-wrap order, so mt evacs straight into m_all f32 (no
    transpose DMA); one mt PSUM tile spans two tiles so the evac is one op.
  - Scatter: edges host-sorted by target; processing order = 16 blocks of
    1024 edges by (sorted_pos % 16); same-target edges are consecutive in
    sorted order so each 1024-token dma_scatter_add has unique targets (max
    in-degree <= 16; duplicate indices WITHIN a call race).  All calls
    scatter-add f32 into one host-pre-zeroed DRAM accumulator with 64-elem
    f32 rows (256B stride, a scatter-add constraint); Tile serializes the
    WAW chain so cross-call duplicates are safe.
  - Final: out (4096, 32) f32 = one strided DRAM->DRAM copy of acc[:, 0:32].
"""
import sys

sys.path.insert(0, "/opt/trn_rl_repo")

import numpy as np

from concourse import bacc, bass, mybir, tile, bass_utils

# problem constants (hardcoded per spec)
B = 8
N = 4096
E = 16384
D = 32          # atom dim
KB = 64         # bond dim
TIL = 1024      # edges per pipeline tile (= scatter block)
NT = E // TIL   # 16 tiles
CH = 8          # (j,i) chunks per tile
NBLK = 16       # sorted-mod blocks (requires max in-degree <= NBLK)
TPB = E // NBLK  # tokens per block = 1024
F32 = mybir.dt.float32
BF16 = mybir.dt.bfloat16
I16 = mybir.dt.int16

_PROGRAM_CACHE = {}

# tunables
CFG = dict(
    til=512,         # edges per pipeline tile
    route="DGADAGDA",  # per-chunk multiply route (even tiles):
                       # A=ACT-evac+DVE-tt(2x), D=DVE-tt-from-PSUM,
                       # G=ACT-evac+Pool-tt (Pool: no PSUM access)
    route2="ADDGDGAD",  # odd-tile route (ACT/DVE load balancing)
    bw_bufs=6,       # PSUM bufs for bw chunk tiles (1 bank each)
    mt_bufs=2,       # PSUM bufs for the mT accumulator (1 bank each)
    pt_bufs=18,      # SBUF bufs for pt (all 8 chunks stay live one extra tile)
    bwsb_bufs=5,     # SBUF bufs for ACT-evacuated bw
    bt_bufs=3,       # SBUF bufs for bondT tiles
    sg_bufs=3,       # SBUF bufs for srcg tiles
    pipe_ahead=2,    # emit create matmul for chunk c+pipe_ahead before sel of c
    tail_at=1,       # chunk index of tile t after which tile t-1's tail is emitted
    warmup=8,        # PE warm-up matmuls (p-state ramp) while DMAs load
)


def _build_program(cfg=None):
    cfg = {**CFG, **(cfg or {})}
    TIL = cfg["til"]
    NT = E // TIL
    nc = bacc.Bacc("TRN2", target_bir_lowering=False, debug=False, num_devices=B)

    bondT_d = nc.dram_tensor("bondT", (KB, E), BF16, kind="ExternalInput")
    srcg_d = nc.dram_tensor("srcg", (128, E), BF16, kind="ExternalInput")
    w2_d = nc.dram_tensor("w2", (KB, CH * 128), BF16, kind="ExternalInput")
    sel_d = nc.dram_tensor("sel", (128, CH * D), BF16, kind="ExternalInput")
    tgtw_d = nc.dram_tensor("tgtw", (128, E // 16), I16, kind="ExternalInput")
    # f32 accumulator rows of 64 (256B stride, a scatter-add constraint)
    acc_d = nc.dram_tensor("acc", (N, 64), F32, kind="ExternalInput")  # pre-zeroed
    out_d = nc.dram_tensor("out", (N, D), F32, kind="ExternalOutput")

    route = cfg["route"]
    route2 = cfg.get("route2") or route
    assert len(route) == CH and set(route) <= {"A", "D", "G"}
    assert len(route2) == CH and set(route2) <= {"A", "D", "G"}

    with tile.TileContext(nc) as tc:
        with tc.tile_pool(name="const", bufs=1) as cp, \
             tc.tile_pool(name="ptp", bufs=cfg["pt_bufs"]) as wp, \
             tc.tile_pool(name="bwsb", bufs=cfg["bwsb_bufs"]) as bp, \
             tc.tile_pool(name="btp", bufs=cfg["bt_bufs"]) as btp, \
             tc.tile_pool(name="sgp", bufs=cfg["sg_bufs"]) as sgp, \
             tc.tile_pool(name="finp", bufs=3) as fp, \
             tc.tile_pool(name="bwps", bufs=cfg["bw_bufs"], space="PSUM") as bwp, \
             tc.tile_pool(name="mtps", bufs=cfg["mt_bufs"], space="PSUM") as mtp:

            # ---- PE warm-up: garbage matmuls with no input deps keep the PE
            # p-state ramp going while the first DMAs load (never read back) ----
            if cfg["warmup"]:
                wm_in = cp.tile([KB, 512], BF16, name="wm_in")
                nc.gpsimd.memset(wm_in[:], 0.0)
                wm_ps = bwp.tile([128, 512], F32, tag="bw", name="wm_ps")
                for w in range(cfg["warmup"]):
                    nc.tensor.matmul(
                        out=wm_ps[:], lhsT=wm_in[:, 0:128], rhs=wm_in[:],
                        start=True, stop=True, skip_group_check=True,
                    )

            # ---- setup; first-create dependencies first: chunk 0 of w2,
            # then tile-0 bond, then the rest ----
            w2_sb = cp.tile([KB, CH * 128], BF16)
            nc.sync.dma_start(w2_sb[:, 0:128], w2_d.ap()[:, 0:128])

            st = {}
            nq = TIL // 128

            def emit_loads(t):
                esl = slice(t * TIL, (t + 1) * TIL)
                bt_sb = btp.tile([KB, TIL], BF16, tag="bt", name="bt_sb")
                nc.sync.dma_start(bt_sb[:], bondT_d.ap()[:, esl])
                sg_sb = sgp.tile([128, TIL], BF16, tag="sg", name="sg_sb")
                nc.sync.dma_start(sg_sb[:], srcg_d.ap()[:, esl])
                st[t] = dict(bt=bt_sb, sg=sg_sb, bw={}, pt={}, mt=None)

            emit_loads(0)

            nc.sync.dma_start(w2_sb[:, 128:], w2_d.ap()[:, 128:])
            sel_sb = cp.tile([128, CH * D], BF16)
            nc.sync.dma_start(sel_sb[:], sel_d.ap())
            tgtw_sb = cp.tile([128, E // 16], I16)
            nc.scalar.dma_start(tgtw_sb[:], tgtw_d.ap())

            # edge-major f32 messages, token-wrapped: token q at [q%128, q//128, :]
            m_all = cp.tile([128, E // 128, D], F32)

            ahead = max(cfg["pipe_ahead"], 0)

            def emit_create(t, c):
                s = st[t]
                bw_ps = bwp.tile([128, TIL], F32, tag="bw", name="bw_ps")
                for h in range(TIL // 512):
                    hs = slice(h * 512, (h + 1) * 512)
                    nc.tensor.matmul(
                        out=bw_ps[:, hs],
                        lhsT=w2_sb[:, c * 128:(c + 1) * 128],
                        rhs=s["bt"][:, hs],
                        start=True, stop=True,
                    )
                s["bw"][c] = bw_ps

            def emit_mult(t, c):
                s = st[t]
                r = (route if t % 2 == 0 else route2)[c]
                bw_ps = s["bw"].pop(c)
                pt_sb = wp.tile([128, TIL], BF16, tag="pt", name="pt_sb")
                if r in "AG":
                    bw_sb = bp.tile([128, TIL], BF16, tag="bwsb", name="bw_sb")
                    nc.scalar.copy(bw_sb[:], bw_ps[:])
                    eng = nc.vector if r == "A" else nc.gpsimd
                    eng.tensor_tensor(
                        out=pt_sb[:], in0=bw_sb[:], in1=s["sg"][:],
                        op=mybir.AluOpType.mult,
                    )
                else:
                    nc.vector.tensor_tensor(
                        out=pt_sb[:], in0=bw_ps[:], in1=s["sg"][:],
                        op=mybir.AluOpType.mult,
                    )
                s["pt"][c] = (pt_sb, 0)

            mts = {}

            def emit_sel(t, b):
                # weight-stationary j-reduce for 128-edge block b of tile t:
                # lhsT is the product block (128 partitions = (i,j), 128 free =
                # edges), rhs the 32-wide selector; out partition = edge-local
                # index.  The 8 chunk matmuls of a block run CONSECUTIVELY:
                # interleaved PSUM accumulation groups are miscompiled.
                # One mt PSUM tile spans TWO tiles so the evac is one ACT op.
                s = st[t]
                if t % 2 == 0 and b == 0:
                    mts[t // 2] = mtp.tile([128, 2 * nq * D], F32, tag="mt",
                                           name="mt_ps")
                mt = mts[t // 2]
                off = ((t % 2) * nq + b) * D
                for c in range(CH):
                    pt_sb, poff = s["pt"][c]
                    ap = pt_sb[:]
                    if len(pt_sb.shape) == 3:
                        ap = ap.rearrange("p two x -> p (two x)")
                    nc.tensor.matmul(
                        out=mt[:, off:off + D],
                        lhsT=ap[:, poff + b * 128:poff + (b + 1) * 128],
                        rhs=sel_sb[:, c * D:(c + 1) * D],
                        start=(c == 0), stop=(c == CH - 1),
                        skip_group_check=True,
                    )
                if b == nq - 1:
                    s["pt"].clear()

            # scatter batch: 1024 tokens per dma_scatter_add (amortizes the
            # ~1us SWDGE fixed overhead).  Must not exceed 1024: a call may
            # not span two mod-16 interleave blocks (duplicate targets race).
            scb = max(1024 // TIL, 1)

            def emit_tail(t, last=False):
                if t % 2 == 1:
                    mt = mts.pop(t // 2)
                    sl0 = (t - 1) * nq
                    mv = m_all[:, sl0:sl0 + 2 * nq, :].rearrange("p s i -> p (s i)")
                    if last:
                        # halve the exposed evac latency: ACT + DVE in parallel
                        half = nq * D
                        nc.scalar.copy(mv[:, 0:half], mt[:, 0:half])
                        nc.vector.tensor_copy(mv[:, half:], mt[:, half:])
                    else:
                        nc.scalar.copy(mv, mt[:])
                if (t + 1) % scb == 0:
                    t0 = t + 1 - scb
                    ntok = scb * TIL
                    nc.gpsimd.dma_scatter_add(
                        out_ap=acc_d.ap()[:, 0:D],
                        in_ap=m_all[:, t0 * nq:(t + 1) * nq, :],
                        idxs_ap=tgtw_sb[:, t0 * (TIL // 16):(t + 1) * (TIL // 16)],
                        num_idxs=ntok,
                        num_idxs_reg=ntok,
                        elem_size=D,
                        elem_step=64,
                    )
                del st[t]

            if cfg.get("sel_early"):
                sel_at = {(b * CH) // nq: b for b in range(nq)}
            else:
                sel_at = {((b + 1) * CH) // nq - 1: b for b in range(nq)}
            for t in range(NT):
                if t + 1 < NT:
                    emit_loads(t + 1)
                for c in range(CH):
                    emit_create(t, c)
                    emit_mult(t, c)
                    if t >= 1 and c in sel_at:
                        emit_sel(t - 1, sel_at[c])
                if t >= 1:
                    emit_tail(t - 1)
            for b in range(nq):
                emit_sel(NT - 1, b)
            emit_tail(NT - 1, last=True)

            # ---- final: the accumulator IS the f32 output (narrower rows):
            # one DRAM->DRAM strided copy, no SBUF staging, no convert ----
            nc.sync.dma_start(out_d.ap(), acc_d.ap()[:, 0:D])

    nc.compile()
    return nc


def _host_prep(atom_state, bond_state, bond_transform, connectivity):
    """Build per-core input maps. Pure layout / index-metadata / dtype prep."""
    import ml_dtypes

    W = np.asarray(bond_transform, dtype=np.float32)  # (KB, D*D)

    # W2[k, c*128 + p] = W[k, (4c + p%4)*D + p//4]   (i = 4c + p%4, j = p//4)
    p = np.arange(128)
    cc = np.arange(CH)
    i_idx = 4 * cc[:, None] + (p % 4)[None, :]   # (CH, 128)
    j_idx = np.broadcast_to((p // 4)[None, :], (CH, 128))
    w2 = W[:, (i_idx * D + j_idx).reshape(-1)].astype(ml_dtypes.bfloat16)

    # selectors S_c[p, m] = [4c + p%4 == m]
    sel = np.zeros((128, CH * D), dtype=np.float32)
    for c in range(CH):
        sel[p, c * D + 4 * c + (p % 4)] = 1.0
    sel_bf = sel.astype(ml_dtypes.bfloat16)

    zeros_acc = np.zeros((N, 64), dtype=np.float32)

    in_maps = []
    for b in range(B):
        src = np.asarray(connectivity[b, :, 0], dtype=np.int64)
        tgt = np.asarray(connectivity[b, :, 1], dtype=np.int64)
        order = np.argsort(tgt, kind="stable")
        deg = np.bincount(tgt, minlength=N).max()
        if deg > NBLK:
            raise ValueError(f"max in-degree {deg} exceeds {NBLK}")
        # processing order: blocks by sorted_pos % NBLK
        proc = np.concatenate([order[c::NBLK] for c in range(NBLK)])
        tgtp = tgt[proc].astype(np.int16)

        bondT = np.ascontiguousarray(
            np.asarray(bond_state[b], dtype=np.float32).T[:, proc]
        ).astype(ml_dtypes.bfloat16)  # (KB, E)

        # srcg[p, e] = atom[src[proc[e]], p//4]
        atomg = np.asarray(atom_state[b], dtype=np.float32)[src[proc]]  # (E, D)
        srcg = np.ascontiguousarray(
            np.repeat(atomg.T.astype(ml_dtypes.bfloat16), 4, axis=0)
        )  # (128, E)

        # wrapped idx table: idxs[p, s] = vals[16*s + p%16], tiled to 128 partitions
        def wrap16(vals):
            w = vals.reshape(-1, 16).T  # (16, E//16)
            return np.ascontiguousarray(np.tile(w, (8, 1)), dtype=np.int16)

        in_maps.append({
            "bondT": bondT,
            "srcg": srcg,
            "w2": w2,
            "sel": sel_bf,
            "tgtw": wrap16(tgtp),
            "acc": zeros_acc,
        })
    return in_maps


def kernel(atom_state, bond_state, bond_transform, connectivity):
    if "nc" not in _PROGRAM_CACHE:
        _PROGRAM_CACHE["nc"] = _build_program()
    nc = _PROGRAM_CACHE["nc"]

    in_maps = _host_prep(atom_state, bond_state, bond_transform, connectivity)
    res = bass_utils.run_bass_kernel_spmd(nc, in_maps, list(range(B)))
    out = np.stack([res.results[b]["out"] for b in range(B)], axis=0)
    return out.astype(np.float32)


# revision 51
# speedup vs baseline: 1.0007x; 1.0007x over previous
"""Trainium2 Bass kernel for nn_BondMatrixMessage (GNN bond-matrix message passing).

Per batch b (one NeuronCore each, B=8 => 8 cores):
    bw[e,(i,j)] = sum_k bond[e,k] * W[k,(i,j)]          (PE, bf16)
    m[e,i]      = sum_j bw[e,(i,j)] * atom[src[e],j]    (evac/mult + PE j-reduce)
    out[t,:]    = sum_{e: tgt[e]=t} m[e,:]              (sorted-edge scatter-add)

Feature-major chunked layout: per 512-edge tile, 8 chunks of 128 partitions,
chunk c partition p <-> (i = 4c + p%4, j = p//4).
  - srcg[p, e] = atom[src[e], p//4] is host-prepared (pure index/layout prep,
    same class as the host edge sort) and DMA-streamed per tile.
  - bw_c = W2_c^T @ bondT_tile (PSUM fp32, one 512-col matmul per chunk).
  - multiply route per chunk, alternating per tile parity (route/route2) to
    balance engines: 'A' = ACT-evac to SBUF bf16 then DVE tensor_tensor (2x
    mode); 'D' = DVE tensor_tensor direct from PSUM (1x); 'G' = ACT-evac +
    Pool tensor_tensor (GPSIMD cannot access PSUM).
  - j-reduction as weight-stationary matmuls: for each 128-edge block b,
    lhsT = pt_c[:, b*128:(b+1)*128] (the multiplied products as PE weights),
    rhs = sel_c (128, 32) with sel_c[p, m] = [4c + p%4 == m]:
    mt[e_local, i] += sum_p pt[p, e]*sel[p, i].  The 8 chunk matmuls of a
    block accumulate into PSUM CONSECUTIVELY (interleaved accumulation groups
    are miscompiled).  Output partition = token index mod 128 == the
    scatter's token-wrap order, so mt evacs straight into m_all f32 (no
    transpose DMA); one mt PSUM tile spans two tiles so the evac is one op.
  - Scatter: edges host-sorted by target; processing order = 16 blocks of
    1024 edges by (sorted_pos % 16); same-target edges are consecutive in
    sorted order so each 1024-token dma_scatter_add has unique targets (max
    in-degree <= 16; duplicate indices WITHIN a call race).  All calls
    scatter-add f32 into one host-pre-zeroed DRAM accumulator with 64-elem
    f32 rows (256B stride, a scatter-add constraint); Tile serializes the
    WAW chain so cross-call duplicates are safe.
  - Final: out (4096, 32) f32 = one strided DRAM->DRAM copy of acc[:, 0:32].
"""
import sys

sys.path.insert(0, "/opt/trn_rl_repo")

import numpy as np

from concourse import bacc, bass, mybir, tile, bass_utils

# problem constants (hardcoded per spec)
B = 8
N = 4096
E = 16384
D = 32          # atom dim
KB = 64         # bond dim
TIL = 1024      # edges per pipeline tile (= scatter block)
NT = E // TIL   # 16 tiles
CH = 8          # (j,i) chunks per tile
NBLK = 16       # sorted-mod blocks (requires max in-degree <= NBLK)
TPB = E // NBLK  # tokens per block = 1024
F32 = mybir.dt.float32
BF16 = mybir.dt.bfloat16
I16 = mybir.dt.int16

_PROGRAM_CACHE = {}

# tunables
CFG = dict(
    til=512,         # edges per pipeline tile
    route="DGADAGDA",  # per-chunk multiply route (even tiles):
                       # A=ACT-evac+DVE-tt(2x), D=DVE-tt-from-PSUM,
                       # G=ACT-evac+Pool-tt (Pool: no PSUM access)
    route2="ADDGDAGD",  # odd-tile route (ACT/DVE load balancing)
    bw_bufs=6,       # PSUM bufs for bw chunk tiles (1 bank each)
    mt_bufs=2,       # PSUM bufs for the mT accumulator (1 bank each)
    pt_bufs=18,      # SBUF bufs for pt (all 8 chunks stay live one extra tile)
    bwsb_bufs=5,     # SBUF bufs for ACT-evacuated bw
    bt_bufs=3,       # SBUF bufs for bondT tiles
    sg_bufs=3,       # SBUF bufs for srcg tiles
    pipe_ahead=2,    # emit create matmul for chunk c+pipe_ahead before sel of c
    tail_at=1,       # chunk index of tile t after which tile t-1's tail is emitted
    warmup=8,        # PE warm-up matmuls (p-state ramp) while DMAs load
)


def _build_program(cfg=None):
    cfg = {**CFG, **(cfg or {})}
    TIL = cfg["til"]
    NT = E // TIL
    nc = bacc.Bacc("TRN2", target_bir_lowering=False, debug=False, num_devices=B)

    bondT_d = nc.dram_tensor("bondT", (KB, E), BF16, kind="ExternalInput")
    srcg_d = nc.dram_tensor("srcg", (128, E), BF16, kind="ExternalInput")
    w2_d = nc.dram_tensor("w2", (KB, CH * 128), BF16, kind="ExternalInput")
    sel_d = nc.dram_tensor("sel", (128, CH * D), BF16, kind="ExternalInput")
    tgtw_d = nc.dram_tensor("tgtw", (128, E // 16), I16, kind="ExternalInput")
    # f32 accumulator rows of 64 (256B stride, a scatter-add constraint)
    acc_d = nc.dram_tensor("acc", (N, 64), F32, kind="ExternalInput")  # pre-zeroed
    out_d = nc.dram_tensor("out", (N, D), F32, kind="ExternalOutput")

    route = cfg["route"]
    route2 = cfg.get("route2") or route
    assert len(route) == CH and set(route) <= {"A", "D", "G"}
    assert len(route2) == CH and set(route2) <= {"A", "D", "G"}

    with tile.TileContext(nc) as tc:
        with tc.tile_pool(name="const", bufs=1) as cp, \
             tc.tile_pool(name="ptp", bufs=cfg["pt_bufs"]) as wp, \
             tc.tile_pool(name="bwsb", bufs=cfg["bwsb_bufs"]) as bp, \
             tc.tile_pool(name="btp", bufs=cfg["bt_bufs"]) as btp, \
             tc.tile_pool(name="sgp", bufs=cfg["sg_bufs"]) as sgp, \
             tc.tile_pool(name="finp", bufs=3) as fp, \
             tc.tile_pool(name="bwps", bufs=cfg["bw_bufs"], space="PSUM") as bwp, \
             tc.tile_pool(name="mtps", bufs=cfg["mt_bufs"], space="PSUM") as mtp:

            # ---- PE warm-up: garbage matmuls with no input deps keep the PE
            # p-state ramp going while the first DMAs load (never read back) ----
            if cfg["warmup"]:
                wm_in = cp.tile([KB, 512], BF16, name="wm_in")
                nc.gpsimd.memset(wm_in[:], 0.0)
                wm_ps = bwp.tile([128, 512], F32, tag="bw", name="wm_ps")
                for w in range(cfg["warmup"]):
                    nc.tensor.matmul(
                        out=wm_ps[:], lhsT=wm_in[:, 0:128], rhs=wm_in[:],
                        start=True, stop=True, skip_group_check=True,
                    )

            # ---- setup; first-create dependencies first: chunk 0 of w2,
            # then tile-0 bond, then the rest ----
            w2_sb = cp.tile([KB, CH * 128], BF16)
            nc.sync.dma_start(w2_sb[:, 0:128], w2_d.ap()[:, 0:128])

            st = {}
            nq = TIL // 128

            def emit_loads(t):
                esl = slice(t * TIL, (t + 1) * TIL)
                bt_sb = btp.tile([KB, TIL], BF16, tag="bt", name="bt_sb")
                nc.sync.dma_start(bt_sb[:], bondT_d.ap()[:, esl])
                sg_sb = sgp.tile([128, TIL], BF16, tag="sg", name="sg_sb")
                nc.sync.dma_start(sg_sb[:], srcg_d.ap()[:, esl])
                st[t] = dict(bt=bt_sb, sg=sg_sb, bw={}, pt={}, mt=None)

            emit_loads(0)

            nc.sync.dma_start(w2_sb[:, 128:], w2_d.ap()[:, 128:])
            sel_sb = cp.tile([128, CH * D], BF16)
            nc.sync.dma_start(sel_sb[:], sel_d.ap())
            tgtw_sb = cp.tile([128, E // 16], I16)
            nc.scalar.dma_start(tgtw_sb[:], tgtw_d.ap())

            # edge-major f32 messages, token-wrapped: token q at [q%128, q//128, :]
            m_all = cp.tile([128, E // 128, D], F32)

            ahead = max(cfg["pipe_ahead"], 0)

            def emit_create(t, c):
                s = st[t]
                bw_ps = bwp.tile([128, TIL], F32, tag="bw", name="bw_ps")
                for h in range(TIL // 512):
                    hs = slice(h * 512, (h + 1) * 512)
                    nc.tensor.matmul(
                        out=bw_ps[:, hs],
                        lhsT=w2_sb[:, c * 128:(c + 1) * 128],
                        rhs=s["bt"][:, hs],
                        start=True, stop=True,
                    )
                s["bw"][c] = bw_ps

            def emit_mult(t, c):
                s = st[t]
                r = (route if t % 2 == 0 else route2)[c]
                bw_ps = s["bw"].pop(c)
                pt_sb = wp.tile([128, TIL], BF16, tag="pt", name="pt_sb")
                if r in "AG":
                    bw_sb = bp.tile([128, TIL], BF16, tag="bwsb", name="bw_sb")
                    nc.scalar.copy(bw_sb[:], bw_ps[:])
                    eng = nc.vector if r == "A" else nc.gpsimd
                    eng.tensor_tensor(
                        out=pt_sb[:], in0=bw_sb[:], in1=s["sg"][:],
                        op=mybir.AluOpType.mult,
                    )
                else:
                    nc.vector.tensor_tensor(
                        out=pt_sb[:], in0=bw_ps[:], in1=s["sg"][:],
                        op=mybir.AluOpType.mult,
                    )
                s["pt"][c] = (pt_sb, 0)

            mts = {}

            def emit_sel(t, b):
                # weight-stationary j-reduce for 128-edge block b of tile t:
                # lhsT is the product block (128 partitions = (i,j), 128 free =
                # edges), rhs the 32-wide selector; out partition = edge-local
                # index.  The 8 chunk matmuls of a block run CONSECUTIVELY:
                # interleaved PSUM accumulation groups are miscompiled.
                # One mt PSUM tile spans TWO tiles so the evac is one ACT op.
                s = st[t]
                if t % 2 == 0 and b == 0:
                    mts[t // 2] = mtp.tile([128, 2 * nq * D], F32, tag="mt",
                                           name="mt_ps")
                mt = mts[t // 2]
                off = ((t % 2) * nq + b) * D
                for c in range(CH):
                    pt_sb, poff = s["pt"][c]
                    ap = pt_sb[:]
                    if len(pt_sb.shape) == 3:
                        ap = ap.rearrange("p two x -> p (two x)")
                    nc.tensor.matmul(
                        out=mt[:, off:off + D],
                        lhsT=ap[:, poff + b * 128:poff + (b + 1) * 128],
                        rhs=sel_sb[:, c * D:(c + 1) * D],
                        start=(c == 0), stop=(c == CH - 1),
                        skip_group_check=True,
                    )
                if b == nq - 1:
                    s["pt"].clear()

            # scatter batch: 1024 tokens per dma_scatter_add (amortizes the
            # ~1us SWDGE fixed overhead).  Must not exceed 1024: a call may
            # not span two mod-16 interleave blocks (duplicate targets race).
            scb = max(1024 // TIL, 1)

            def emit_tail(t, last=False):
                if t % 2 == 1:
                    mt = mts.pop(t // 2)
                    sl0 = (t - 1) * nq
                    mv = m_all[:, sl0:sl0 + 2 * nq, :].rearrange("p s i -> p (s i)")
                    if last:
                        # halve the exposed evac latency: ACT + DVE in parallel
                        half = nq * D
                        nc.scalar.copy(mv[:, 0:half], mt[:, 0:half])
                        nc.vector.tensor_copy(mv[:, half:], mt[:, half:])
                    else:
                        nc.scalar.copy(mv, mt[:])
                if (t + 1) % scb == 0:
                    t0 = t + 1 - scb
                    ntok = scb * TIL
                    nc.gpsimd.dma_scatter_add(
                        out_ap=acc_d.ap()[:, 0:D],
                        in_ap=m_all[:, t0 * nq:(t + 1) * nq, :],
                        idxs_ap=tgtw_sb[:, t0 * (TIL // 16):(t + 1) * (TIL // 16)],
                        num_idxs=ntok,
                        num_idxs_reg=ntok,
                        elem_size=D,
                        elem_step=64,
                    )
                del st[t]

            if cfg.get("sel_early"):
                sel_at = {(b * CH) // nq: b for b in range(nq)}
            else:
                sel_at = {((b + 1) * CH) // nq - 1: b for b in range(nq)}
            for t in range(NT):
                if t + 1 < NT:
                    emit_loads(t + 1)
                for c in range(CH):
                    emit_create(t, c)
                    emit_mult(t, c)
                    if t >= 1 and c in sel_at:
                        emit_sel(t - 1, sel_at[c])
                if t >= 1:
                    emit_tail(t - 1)
            for b in range(nq):
                emit_sel(NT - 1, b)
            emit_tail(NT - 1, last=True)

            # ---- final: the accumulator IS the f32 output (narrower rows):
            # one DRAM->DRAM strided copy, no SBUF staging, no convert ----
            nc.sync.dma_start(out_d.ap(), acc_d.ap()[:, 0:D])

    nc.compile()
    return nc


def _host_prep(atom_state, bond_state, bond_transform, connectivity):
    """Build per-core input maps. Pure layout / index-metadata / dtype prep."""
    import ml_dtypes

    W = np.asarray(bond_transform, dtype=np.float32)  # (KB, D*D)

    # W2[k, c*128 + p] = W[k, (4c + p%4)*D + p//4]   (i = 4c + p%4, j = p//4)
    p = np.arange(128)
    cc = np.arange(CH)
    i_idx = 4 * cc[:, None] + (p % 4)[None, :]   # (CH, 128)
    j_idx = np.broadcast_to((p // 4)[None, :], (CH, 128))
    w2 = W[:, (i_idx * D + j_idx).reshape(-1)].astype(ml_dtypes.bfloat16)

    # selectors S_c[p, m] = [4c + p%4 == m]
    sel = np.zeros((128, CH * D), dtype=np.float32)
    for c in range(CH):
        sel[p, c * D + 4 * c + (p % 4)] = 1.0
    sel_bf = sel.astype(ml_dtypes.bfloat16)

    zeros_acc = np.zeros((N, 64), dtype=np.float32)

    in_maps = []
    for b in range(B):
        src = np.asarray(connectivity[b, :, 0], dtype=np.int64)
        tgt = np.asarray(connectivity[b, :, 1], dtype=np.int64)
        order = np.argsort(tgt, kind="stable")
        deg = np.bincount(tgt, minlength=N).max()
        if deg > NBLK:
            raise ValueError(f"max in-degree {deg} exceeds {NBLK}")
        # processing order: blocks by sorted_pos % NBLK
        proc = np.concatenate([order[c::NBLK] for c in range(NBLK)])
        tgtp = tgt[proc].astype(np.int16)

        bondT = np.ascontiguousarray(
            np.asarray(bond_state[b], dtype=np.float32).T[:, proc]
        ).astype(ml_dtypes.bfloat16)  # (KB, E)

        # srcg[p, e] = atom[src[proc[e]], p//4]
        atomg = np.asarray(atom_state[b], dtype=np.float32)[src[proc]]  # (E, D)
        srcg = np.ascontiguousarray(
            np.repeat(atomg.T.astype(ml_dtypes.bfloat16), 4, axis=0)
        )  # (128, E)

        # wrapped idx table: idxs[p, s] = vals[16*s + p%16], tiled to 128 partitions
        def wrap16(vals):
            w = vals.reshape(-1, 16).T  # (16, E//16)
            return np.ascontiguousarray(np.tile(w, (8, 1)), dtype=np.int16)

        in_maps.append({
            "bondT": bondT,
            "srcg": srcg,
            "w2": w2,
            "sel": sel_bf,
            "tgtw": wrap16(tgtp),
            "acc": zeros_acc,
        })
    return in_maps


def kernel(atom_state, bond_state, bond_transform, connectivity):
    if "nc" not in _PROGRAM_CACHE:
        _PROGRAM_CACHE["nc"] = _build_program()
    nc = _PROGRAM_CACHE["nc"]

    in_maps = _host_prep(atom_state, bond_state, bond_transform, connectivity)
    res = bass_utils.run_bass_kernel_spmd(nc, in_maps, list(range(B)))
    out = np.stack([res.results[b]["out"] for b in range(B)], axis=0)
    return out.astype(np.float32)
